# revision 1
# baseline (speedup 1.0000x reference)
"""NURBS surface evaluation on 8 TRN2 NeuronCores.

Reformulation: the reference einsum
    out[x, y, d] = sum_{l,r} bx[l,x] * cp[ix(l,x), iy(r,y), d] * by[r,y]
is a pair of dense matmuls once the 4 basis weights per eval point are
scattered into dense (1024, 32) basis matrices A (x axis) and B (y axis):
    out[:, :, d] = A @ cp[:, :, d] @ B.T
The per-axis span/basis computation (Cox-de-Boor over 1024 points, 36 knots)
is tiny and runs on host; the device does the million-point evaluation:
    stage 1: C[i, 3y+d] = sum_j cp[i,j,d] * B[y,j]      (32x3072, channel-interleaved)
    stage 2: out[x, 3y+d] = sum_i A[x,i] * C[i, 3y+d]   (128x3072 per core)
Sharding: eval-grid x axis split across 8 cores (128 rows each); B / cp
replicated, per-core slice of A^T. Output gathered on host.
"""

import numpy as np

DEGREE = 3
NCTRL = 32
EOUT = 1024
DIM = 3
EPS = 1e-5
NCORES = 8
ROWS = EOUT // NCORES          # 128 eval rows per core
OUTW = EOUT * DIM              # 3072 interleaved output columns


# ----------------------------------------------------------------- host math
def _normalize_knots(kv):
    kv = np.cumsum(np.where(kv < 0.0, np.float32(1e-4), kv), axis=1,
                   dtype=np.float32)
    return (kv - kv[:, :1]) / (kv[:, -1:] - kv[:, :1])


def _find_spans(ev, kv):
    internal = kv[:, DEGREE:-DEGREE]                      # (S, Ki)
    diff = ev[None, None, :] - internal[:, :, None]       # (S, Ki, E)
    diff = np.where(diff > 1e-8, diff, np.float32(1.0))
    return np.argmin(diff, axis=1) + DEGREE               # (S, E)


def _basis(ev, kv, spans):
    # Cox-de-Boor recursion, mirrors the reference op-for-op in f32.
    S, E = spans.shape
    basis = [np.zeros((S, E), kv.dtype) for _ in range(DEGREE + 1)]
    basis[0] = np.ones((S, E), kv.dtype)
    for k in range(1, DEGREE + 1):
        saved = np.zeros((S, E), kv.dtype)
        for r in range(k):
            left = np.take_along_axis(kv, spans + r + 1, axis=1)
            right = np.take_along_axis(kv, spans + 1 - k + r, axis=1)
            denom = (left - ev) + (ev - right)
            safe = np.where(denom == 0.0, np.float32(1.0), denom)
            temp = np.where(denom == 0.0, np.float32(1e-4), basis[r] / safe)
            basis[r] = saved + (left - ev) * temp
            saved = (ev - right) * temp
        basis[k] = saved
    return np.stack(basis, axis=1)                        # (S, DEGREE+1, E)


def _dense_basis_matrix(knots):
    """(EOUT, NCTRL) dense basis matrix M with M[e, i] the weight of control
    index i at eval point e, replicating the reference gather indices
    (span - 3 - l, wrapped once for negatives)."""
    ev = np.linspace(EPS, 1.0 - EPS, EOUT, dtype=np.float32)
    kv = _normalize_knots(np.asarray(knots, dtype=np.float32))
    spans = _find_spans(ev, kv)
    b = _basis(ev, kv, spans)[0]                          # (DEGREE+1, E)
    sp = spans[0]
    M = np.zeros((EOUT, NCTRL), dtype=np.float32)
    for l in range(DEGREE + 1):
        idx = sp - (DEGREE + l)
        idx = np.where(idx < 0, idx + NCTRL, idx)
        M[np.arange(EOUT), idx] = b[l]
    return M


def _round_fp32r(x):
    """Round f32 to fp32r (11 explicit mantissa bits, low 12 bits zero),
    round-to-nearest-even -- matches walrus fp32_to_fp32r."""
    u = np.ascontiguousarray(x, dtype=np.float32).view(np.uint32)
    low = u & np.uint32(0xFFF)
    base = u & np.uint32(0xFFFFF000)
    lsb = (u >> np.uint32(12)) & np.uint32(1)
    inc = (low > 0x800) | ((low == 0x800) & (lsb == 1))
    out = base + inc.astype(np.uint32) * np.uint32(0x1000)
    return out.view(np.float32)


# ------------------------------------------------------------- device kernel
_NC_CACHE = {}


def _build_nc():
    if "nc" in _NC_CACHE:
        return _NC_CACHE["nc"]
    from contextlib import ExitStack
    import concourse.bacc as bacc
    import concourse.tile as tile
    import concourse.mybir as mybir

    f32 = mybir.dt.float32
    f32r = mybir.dt.float32r
    # Bacc (not plain Bass): its finalize() runs generate_event_semaphores,
    # which splits multi-sem waits into EventSemaphore chains -- TRN2 allows
    # at most 1 wait per instruction and the Tile tail drain accumulates one
    # wait per ticked semaphore.
    nc = bacc.Bacc()
    # inputs: [bt | cpt] packed on 32 partitions, plus the A-tile replicated
    # 3x on 96 partitions so stage-2 lhsT/rhs share a base partition.
    # Declared float32r (host pre-rounds to 11 mantissa bits): fp32r matmul is
    # single-pass on the PE (4x the fp32 rate at moving dim >=256).
    INW = EOUT + DIM * NCTRL                              # 1024+96 = 1120
    in_d = nc.declare_dram_parameter("inp", [NCTRL, INW], f32r, isOutput=False)
    at3_d = nc.declare_dram_parameter("at3", [DIM * NCTRL, ROWS], f32r,
                                      isOutput=False)
    out_d = nc.declare_dram_parameter("out", [ROWS, OUTW], f32, isOutput=True)

    NCH = 4                    # y chunks
    CH = EOUT // NCH           # 256

    with tile.TileContext(nc) as tc, ExitStack() as ctx:
        sb = ctx.enter_context(tc.tile_pool(name="sb", bufs=1))
        ps1 = ctx.enter_context(tc.tile_pool(name="ps1", bufs=2, space="PSUM"))
        ps2 = ctx.enter_context(tc.tile_pool(name="ps2", bufs=4, space="PSUM"))

        # Warmups, dependency-free so the scheduler runs them while the input
        # DMA is in flight: the first ACT copy otherwise pays a ~1.3us
        # activation-table load on the critical path, and the PE clock ramps
        # with sustained use (p-state). Results are never read.
        wps = ctx.enter_context(tc.tile_pool(name="warmp", bufs=1, space="PSUM"))
        ws = sb.tile([128, 256], f32, tag="warm")
        nc.gpsimd.memset(ws[:], 0.0)
        nc.scalar.copy(ws[:, 8:16], ws[:, 0:8])
        wp = wps.tile([128, 64], f32, tag="wp")
        nc.tensor.matmul(wp[:], ws[0:32, 0:128], ws[0:32, 0:64])
        nc.tensor.matmul(wp[:], ws[0:32, 0:128], ws[0:32, 0:64])

        inp = sb.tile([NCTRL, INW], f32r, tag="inp")
        at3 = sb.tile([DIM * NCTRL, ROWS], f32r, tag="at3")
        nc.sync.dma_start(inp[:], in_d[:])
        nc.sync.dma_start(at3[:], at3_d[:])
        bt = inp[:, 0:EOUT]
        cpt = inp[:, EOUT:INW]

        # Stage 1: Csep[d*32+i, y] = sum_j cp[i,j,d] * B[y,j].
        # lhsT = cpt [K=32j, M=96(d,i)] does all 3 channels in one matmul.
        # The ACT copy rounds the f32 PSUM result into fp32r for stage 2.
        Csep = sb.tile([DIM * NCTRL, EOUT], f32r, tag="C")
        for h in range(NCH):
            p1 = ps1.tile([DIM * NCTRL, CH], f32, tag="p1")
            nc.tensor.matmul(p1[:], cpt, bt[:, h * CH:(h + 1) * CH])
            # stage-1 copies on ACT, stage-2 on DVE
            nc.scalar.copy(Csep[:, h * CH:(h + 1) * CH], p1[:])

        # Stage 2: out[x, 3y+d] = sum_i A[x,i] * Csep[d*32+i, y]; channel
        # interleave happens in the PSUM->SBUF copy (strided dest is free:
        # PSUM-source copies run 1 elem/cycle regardless).
        out_sb = sb.tile([ROWS, OUTW], f32, tag="osb")
        ov = out_sb[:].rearrange("p (y d) -> p y d", d=DIM)
        W = OUTW // NCH
        for h in range(NCH):
            for d in range(DIM):
                p2 = ps2.tile([ROWS, CH], f32, tag="p2")
                nc.tensor.matmul(
                    p2[:], at3[d * NCTRL:(d + 1) * NCTRL, :],
                    Csep[d * NCTRL:(d + 1) * NCTRL, h * CH:(h + 1) * CH])
                nc.vector.tensor_copy(ov[:, h * CH:(h + 1) * CH, d], p2[:])
            nc.sync.dma_start(out_d[:, h * W:(h + 1) * W],
                              out_sb[:, h * W:(h + 1) * W])

    # Run Bacc's compile pipeline (wait legalization, register allocation)
    # before the BIR is serialized for the compiler.
    nc.finalize()
    _NC_CACHE["nc"] = nc
    return nc


# ------------------------------------------------------------------- wrapper
def _make_in_maps(control_points, knots_x, knots_y):
    cp = np.asarray(control_points, dtype=np.float32)
    A = _dense_basis_matrix(knots_x)                      # (1024, 32)
    B = _dense_basis_matrix(knots_y)                      # (1024, 32)
    At = A.T                                              # (32, 1024) [i, x]
    Bt = B.T                                              # (32, 1024) [j, y]
    # cpt[j, d*32+i] = cp[i, j, d]
    cpt = np.transpose(cp, (1, 2, 0)).reshape(NCTRL, DIM * NCTRL)
    inp = _round_fp32r(np.concatenate([Bt, cpt], axis=1))   # [bt | cpt] layout
    return [
        {
            "inp": inp,
            "at3": _round_fp32r(np.tile(At[:, c * ROWS:(c + 1) * ROWS], (DIM, 1))),
        }
        for c in range(NCORES)
    ]


def kernel(control_points, knots_x, knots_y):
    from concourse.bass_utils import run_bass_kernel_spmd

    in_maps = _make_in_maps(control_points, knots_x, knots_y)
    nc = _build_nc()
    res = run_bass_kernel_spmd(nc, in_maps, core_ids=list(range(NCORES)))
    out = np.concatenate([res.results[c]["out"] for c in range(NCORES)], axis=0)
    return out.reshape(1, EOUT, EOUT, DIM)



# revision 3
# speedup vs baseline: 1.9808x; 1.9808x over previous
"""NURBS surface evaluation on 8 TRN2 NeuronCores — v2.

Math: out[x, y, d] = sum_{i,j} A[x,i] * cp[i,j,d] * B[y,j]
    = sum_j M_d[x, j] * B[y, j],   M_d = A @ cp[:,:,d]
The 1-D basis matrices A, B (1024x32, from Cox-de-Boor over 36 knots) and the
tiny per-core fold M_d^T = cp_d^T @ A_shard^T (32x128x3 per core) are host
precomputation on replicated inputs; the device does the O(Ex*Ey) surface
evaluation: per core, out[x, (d,y)] = M_dT.T @ bt  as 6 matmuls of
[K=32 j] x [128 x, 512 y], fp16 in / f32 PSUM / fp16 out.

Device structure (raw Bacc, no TileContext — no entry barrier / exit drain):
  SP   : one HWDGE DMA of the packed [32, 1408] fp16 input (bt | M)
  PE   : 6 matmuls into 6 PSUM banks
  DVE/ACT/Pool: PSUM->SBUF fp16 copies (spread for throughput)
  Pool : kv_writeback descriptors prepared during the input-DMA window,
         triggered per output third as its copies land (trigger skips the
         HWDGE + DGE-delay latency of a plain DMA issue)
Output DRAM is [128, 3072] fp16, d-major; host converts/reorders to the
(1024, 1024, 3) f32 surface.
"""

import numpy as np

DEGREE = 3
NCTRL = 32
EOUT = 1024
DIM = 3
EPS = 1e-5
NCORES = 8
ROWS = EOUT // NCORES          # 128 eval rows per core
INW = EOUT + DIM * ROWS        # 1024 bt cols + 384 M cols
OUTW = EOUT * DIM              # 3072 output cols per core (d-major)

# chunk c = (d, h): matmul psum[c] = M_dT @ bt[:, 512h:512h+512]
CHUNKS = [(d, h) for d in range(DIM) for h in range(2)]
CW = 512
# PSUM->SBUF copies: only DVE and ACT can read PSUM (walrus rejects GPSIMD
# PSUM access). (engine, chunk, col_off, width) per copy instruction; full
# 512-wide copies amortize the per-instruction PSUM/SBUF access penalty.
COPY_PLAN = [
    ("dve", 0, 0, CW), ("act", 1, 0, CW),
    ("dve", 2, 0, CW), ("act", 3, 0, CW),
    ("dve", 4, 0, CW), ("act", 5, 0, CW),
]


# ----------------------------------------------------------------- host math
def _normalize_knots(kv):
    kv = np.cumsum(np.where(kv < 0.0, np.float32(1e-4), kv), axis=1,
                   dtype=np.float32)
    return (kv - kv[:, :1]) / (kv[:, -1:] - kv[:, :1])


def _find_spans(ev, kv):
    internal = kv[:, DEGREE:-DEGREE]
    diff = ev[None, None, :] - internal[:, :, None]
    diff = np.where(diff > 1e-8, diff, np.float32(1.0))
    return np.argmin(diff, axis=1) + DEGREE


def _basis(ev, kv, spans):
    S, E = spans.shape
    basis = [np.zeros((S, E), kv.dtype) for _ in range(DEGREE + 1)]
    basis[0] = np.ones((S, E), kv.dtype)
    for k in range(1, DEGREE + 1):
        saved = np.zeros((S, E), kv.dtype)
        for r in range(k):
            left = np.take_along_axis(kv, spans + r + 1, axis=1)
            right = np.take_along_axis(kv, spans + 1 - k + r, axis=1)
            denom = (left - ev) + (ev - right)
            safe = np.where(denom == 0.0, np.float32(1.0), denom)
            temp = np.where(denom == 0.0, np.float32(1e-4), basis[r] / safe)
            basis[r] = saved + (left - ev) * temp
            saved = (ev - right) * temp
        basis[k] = saved
    return np.stack(basis, axis=1)


def _dense_basis_matrix(knots):
    ev = np.linspace(EPS, 1.0 - EPS, EOUT, dtype=np.float32)
    kv = _normalize_knots(np.asarray(knots, dtype=np.float32))
    spans = _find_spans(ev, kv)
    b = _basis(ev, kv, spans)[0]
    sp = spans[0]
    M = np.zeros((EOUT, NCTRL), dtype=np.float32)
    for l in range(DEGREE + 1):
        idx = sp - (DEGREE + l)
        idx = np.where(idx < 0, idx + NCTRL, idx)
        M[np.arange(EOUT), idx] = b[l]
    return M


# ------------------------------------------------------------- device kernel
_NC_CACHE = {}


def _build_nc():
    if "nc" in _NC_CACHE:
        return _NC_CACHE["nc"]
    import concourse.bacc as bacc
    import concourse.mybir as mybir
    from concourse.bass import AP

    f16 = mybir.dt.float16
    f32 = mybir.dt.float32
    i32 = mybir.dt.int32

    nc = bacc.Bacc()
    in_d = nc.declare_dram_parameter("inp", [NCTRL, INW], f16, isOutput=False)
    out_d = nc.declare_dram_parameter("out", [ROWS, OUTW], f32, isOutput=True)

    sb_in = nc.alloc_sbuf_tensor("sb_in", [NCTRL, INW], f16)
    sb_out = nc.alloc_sbuf_tensor("sb_out", [ROWS, OUTW], f32)
    idx0 = nc.alloc_sbuf_tensor("idx0", [ROWS, 1], i32)
    ps = [nc.alloc_psum_tensor(f"ps{c}", [ROWS, CW], f32)
          for c in range(len(CHUNKS))]

    s_din = nc.alloc_semaphore("s_din")
    s_mm = nc.alloc_semaphore("s_mm")
    s_copy = {e: nc.alloc_semaphore(f"s_{e}") for e in ("dve", "act", "pool")}
    s_wprep = nc.alloc_semaphore("s_wprep")
    s_dout = nc.alloc_semaphore("s_dout")

    eng = {"dve": nc.vector, "act": nc.scalar, "pool": nc.gpsimd}

    # ---- SP: one packed input DMA (bt | M), fp16
    nc.sync.dma_start(sb_in[:], in_d[:]).then_inc(s_din, 16)

    # ---- Pool: writeback descriptor prep (data-independent, overlaps the
    # input DMA), then per-third triggers gated on the copies.
    nc.gpsimd.memset(idx0[:], 0)
    for k in range(DIM):
        in_ap = sb_out[:, k * EOUT:(k + 1) * EOUT].rearrange(
            "p (o b c) -> p o b c", o=1, b=1)
        base = out_d[:, k * EOUT:(k + 1) * EOUT]
        ap4 = AP(base.tensor, base.offset,
                 [(OUTW * ROWS, 1), (OUTW, ROWS), (OUTW, 1), (1, EOUT)])
        nc.gpsimd.kv_writeback(ap4, in_ap, idx0[:],
                               prepare_only=True, sem=s_dout
                               ).then_inc(s_wprep, 1)

    # plan entry -> (sem, count-within-engine) for the trigger gates
    gate = []
    seen = {e: 0 for e in s_copy}
    for e, c, off, w in COPY_PLAN:
        seen[e] += 1
        gate.append((s_copy[e], seen[e]))

    def emit_copy(i):
        e, c, off, w = COPY_PLAN[i]
        d, h = CHUNKS[c]
        col = d * EOUT + h * CW + off
        eng[e].wait_ge(s_mm, c + 1)
        if e == "act":
            ins = eng[e].copy(sb_out[:, col:col + w], ps[c][:, off:off + w])
        else:
            ins = eng[e].tensor_copy(sb_out[:, col:col + w],
                                     ps[c][:, off:off + w])
        ins.then_inc(s_copy[e], 1)

    def emit_trigger(k):
        nc.gpsimd.wait_ge(s_wprep, k + 1)
        for i, (e, c, off, w) in enumerate(COPY_PLAN):
            if c in (2 * k, 2 * k + 1):
                nc.gpsimd.wait_ge(*gate[i])
        nc.gpsimd.trigger_dma(count=1)

    for k in range(DIM):
        emit_trigger(k)
    nc.gpsimd.wait_ge(s_dout, 16 * DIM)

    # ---- PE: 6 matmuls, chunk c = (d, h). The standalone double wait blocks
    # PE's SEQ until the input lands (~3.1us), so every matmul DISPATCHES
    # after the 3us p-state ramp point and runs at the full 2.4GHz rate.
    nc.tensor.wait_ge(s_din, 16)
    nc.tensor.wait_ge(s_din, 16)
    for c, (d, h) in enumerate(CHUNKS):
        lhsT = sb_in[:, EOUT + ROWS * d: EOUT + ROWS * (d + 1)]   # [32j, 128x]
        rhs = sb_in[:, CW * h: CW * (h + 1)]                      # [32j, 512y]
        nc.tensor.matmul(ps[c][:], lhsT, rhs).then_inc(s_mm, 1)

    # ---- copies on DVE / ACT (per-engine, in plan order)
    for i in range(len(COPY_PLAN)):
        emit_copy(i)

    nc.finalize()
    _NC_CACHE["nc"] = nc
    return nc


# ------------------------------------------------------------------- wrapper
def _make_in_maps(control_points, knots_x, knots_y):
    cp = np.asarray(control_points, dtype=np.float32)
    A = _dense_basis_matrix(knots_x)                      # (1024, 32) [x, i]
    B = _dense_basis_matrix(knots_y)                      # (1024, 32) [y, j]
    Bt = np.ascontiguousarray(B.T)                        # (32, 1024) [j, y]
    maps = []
    for c in range(NCORES):
        Ac = A[c * ROWS:(c + 1) * ROWS]                   # (128, 32) [x, i]
        # MT[j, d*128+x] = sum_i cp[i,j,d] * Ac[x,i]
        MT = np.einsum("ijd,xi->jdx", cp, Ac).reshape(NCTRL, DIM * ROWS)
        inp = np.concatenate([Bt, MT], axis=1).astype(np.float16)
        maps.append({"inp": inp})
    return maps



def kernel(control_points, knots_x, knots_y):
    from concourse.bass_utils import run_bass_kernel_spmd

    in_maps = _make_in_maps(control_points, knots_x, knots_y)
    nc = _build_nc()
    res = run_bass_kernel_spmd(nc, in_maps, core_ids=list(range(NCORES)))
    # per-core out is [128, 3*1024] fp16, d-major; -> (1024, 1024, 3) f32
    out = np.concatenate([np.asarray(res.results[c]["out"])
                          for c in range(NCORES)], axis=0)
    out = out.reshape(EOUT, DIM, EOUT).transpose(0, 2, 1).astype(np.float32)
    return out.reshape(1, EOUT, EOUT, DIM)


# revision 4
# speedup vs baseline: 1.9960x; 1.0076x over previous
"""NURBS surface evaluation on 8 TRN2 NeuronCores — v2.

Math: out[x, y, d] = sum_{i,j} A[x,i] * cp[i,j,d] * B[y,j]
    = sum_j M_d[x, j] * B[y, j],   M_d = A @ cp[:,:,d]
The 1-D basis matrices A, B (1024x32, from Cox-de-Boor over 36 knots) and the
tiny per-core fold M_d^T = cp_d^T @ A_shard^T (32x128x3 per core) are host
precomputation on replicated inputs; the device does the O(Ex*Ey) surface
evaluation: per core, out[x, (d,y)] = M_dT.T @ bt  as 6 matmuls of
[K=32 j] x [128 x, 512 y], fp16 in / f32 PSUM / fp16 out.

Device structure (raw Bacc, no TileContext — no entry barrier / exit drain):
  SP   : one HWDGE DMA of the packed [32, 1408] fp16 input (bt | M)
  PE   : 6 matmuls into 6 PSUM banks
  DVE/ACT/Pool: PSUM->SBUF fp16 copies (spread for throughput)
  Pool : kv_writeback descriptors prepared during the input-DMA window,
         triggered per output third as its copies land (trigger skips the
         HWDGE + DGE-delay latency of a plain DMA issue)
Output DRAM is [128, 3072] fp16, d-major; host converts/reorders to the
(1024, 1024, 3) f32 surface.
"""

import numpy as np

DEGREE = 3
NCTRL = 32
EOUT = 1024
DIM = 3
EPS = 1e-5
NCORES = 8
ROWS = EOUT // NCORES          # 128 eval rows per core
INW = EOUT + DIM * ROWS        # 1024 bt cols + 384 M cols
OUTW = EOUT * DIM              # 3072 output cols per core (d-major)

# chunk c = (d, h): matmul psum[c] = M_dT @ bt[:, 512h:512h+512]
CHUNKS = [(d, h) for d in range(DIM) for h in range(2)]
CW = 512
# PSUM->SBUF copies: only DVE and ACT can read PSUM (walrus rejects GPSIMD
# PSUM access). (engine, chunk, col_off, width) per copy instruction; full
# 512-wide copies amortize the per-instruction PSUM/SBUF access penalty.
COPY_PLAN = [
    ("dve", 0, 0, CW), ("act", 1, 0, CW),
    ("dve", 2, 0, CW), ("act", 3, 0, CW),
    ("dve", 4, 0, CW), ("act", 5, 0, CW),
]


# ----------------------------------------------------------------- host math
def _normalize_knots(kv):
    kv = np.cumsum(np.where(kv < 0.0, np.float32(1e-4), kv), axis=1,
                   dtype=np.float32)
    return (kv - kv[:, :1]) / (kv[:, -1:] - kv[:, :1])


def _find_spans(ev, kv):
    internal = kv[:, DEGREE:-DEGREE]
    diff = ev[None, None, :] - internal[:, :, None]
    diff = np.where(diff > 1e-8, diff, np.float32(1.0))
    return np.argmin(diff, axis=1) + DEGREE


def _basis(ev, kv, spans):
    S, E = spans.shape
    basis = [np.zeros((S, E), kv.dtype) for _ in range(DEGREE + 1)]
    basis[0] = np.ones((S, E), kv.dtype)
    for k in range(1, DEGREE + 1):
        saved = np.zeros((S, E), kv.dtype)
        for r in range(k):
            left = np.take_along_axis(kv, spans + r + 1, axis=1)
            right = np.take_along_axis(kv, spans + 1 - k + r, axis=1)
            denom = (left - ev) + (ev - right)
            safe = np.where(denom == 0.0, np.float32(1.0), denom)
            temp = np.where(denom == 0.0, np.float32(1e-4), basis[r] / safe)
            basis[r] = saved + (left - ev) * temp
            saved = (ev - right) * temp
        basis[k] = saved
    return np.stack(basis, axis=1)


def _dense_basis_matrix(knots):
    ev = np.linspace(EPS, 1.0 - EPS, EOUT, dtype=np.float32)
    kv = _normalize_knots(np.asarray(knots, dtype=np.float32))
    spans = _find_spans(ev, kv)
    b = _basis(ev, kv, spans)[0]
    sp = spans[0]
    M = np.zeros((EOUT, NCTRL), dtype=np.float32)
    for l in range(DEGREE + 1):
        idx = sp - (DEGREE + l)
        idx = np.where(idx < 0, idx + NCTRL, idx)
        M[np.arange(EOUT), idx] = b[l]
    return M


# ------------------------------------------------------------- device kernel
_NC_CACHE = {}


def _build_nc():
    if "nc" in _NC_CACHE:
        return _NC_CACHE["nc"]
    import concourse.bacc as bacc
    import concourse.mybir as mybir
    from concourse.bass import AP

    f16 = mybir.dt.float16
    f32 = mybir.dt.float32
    i32 = mybir.dt.int32

    nc = bacc.Bacc()
    in_d = nc.declare_dram_parameter("inp", [NCTRL, INW], f16, isOutput=False)
    out_d = nc.declare_dram_parameter("out", [ROWS, OUTW], f16, isOutput=True)

    sb_in = nc.alloc_sbuf_tensor("sb_in", [NCTRL, INW], f16)
    sb_out = nc.alloc_sbuf_tensor("sb_out", [ROWS, OUTW], f16)
    idx0 = nc.alloc_sbuf_tensor("idx0", [ROWS, 1], i32)
    ps = [nc.alloc_psum_tensor(f"ps{c}", [ROWS, CW], f32)
          for c in range(len(CHUNKS))]

    s_din = nc.alloc_semaphore("s_din")
    s_mm = nc.alloc_semaphore("s_mm")
    s_copy = {e: nc.alloc_semaphore(f"s_{e}") for e in ("dve", "act", "pool")}
    s_wprep = nc.alloc_semaphore("s_wprep")
    s_dout = nc.alloc_semaphore("s_dout")

    eng = {"dve": nc.vector, "act": nc.scalar, "pool": nc.gpsimd}

    # ---- SP: one packed input DMA (bt | M), fp16
    nc.sync.dma_start(sb_in[:], in_d[:]).then_inc(s_din, 16)

    # ---- Pool: writeback descriptor prep (data-independent, overlaps the
    # input DMA), then triggers gated on the copies. Two writebacks: cols
    # [0, 2048) after chunks 0-3 land, cols [2048, 3072) after chunks 4-5.
    WBS = [(0, 2048, range(4)), (2048, 1024, range(4, 6))]
    nc.gpsimd.memset(idx0[:], 0)
    for col0, w, _ in WBS:
        in_ap = sb_out[:, col0:col0 + w].rearrange(
            "p (o b c) -> p o b c", o=1, b=1)
        base = out_d[:, col0:col0 + w]
        ap4 = AP(base.tensor, base.offset,
                 [(OUTW * ROWS, 1), (OUTW, ROWS), (OUTW, 1), (1, w)])
        nc.gpsimd.kv_writeback(ap4, in_ap, idx0[:],
                               prepare_only=True, sem=s_dout
                               ).then_inc(s_wprep, 1)

    # plan entry -> (sem, count-within-engine) for the trigger gates
    gate = []
    seen = {e: 0 for e in s_copy}
    for e, c, off, w in COPY_PLAN:
        seen[e] += 1
        gate.append((s_copy[e], seen[e]))

    def emit_copy(i):
        e, c, off, w = COPY_PLAN[i]
        d, h = CHUNKS[c]
        col = d * EOUT + h * CW + off
        eng[e].wait_ge(s_mm, c + 1)
        if e == "act":
            ins = eng[e].copy(sb_out[:, col:col + w], ps[c][:, off:off + w])
        else:
            ins = eng[e].tensor_copy(sb_out[:, col:col + w],
                                     ps[c][:, off:off + w])
        ins.then_inc(s_copy[e], 1)

    for k, (col0, w, chunks) in enumerate(WBS):
        nc.gpsimd.wait_ge(s_wprep, k + 1)
        for i, (e, c, off, cw) in enumerate(COPY_PLAN):
            if c in chunks:
                nc.gpsimd.wait_ge(*gate[i])
        nc.gpsimd.trigger_dma(count=1)
    nc.gpsimd.wait_ge(s_dout, 16 * len(WBS))

    # ---- PE: 6 matmuls, chunk c = (d, h). The standalone double wait blocks
    # PE's SEQ until the input lands (~3.1us), so every matmul DISPATCHES
    # after the 3us p-state ramp point and runs at the full 2.4GHz rate.
    nc.tensor.wait_ge(s_din, 16)
    nc.tensor.wait_ge(s_din, 16)
    for c, (d, h) in enumerate(CHUNKS):
        lhsT = sb_in[:, EOUT + ROWS * d: EOUT + ROWS * (d + 1)]   # [32j, 128x]
        rhs = sb_in[:, CW * h: CW * (h + 1)]                      # [32j, 512y]
        nc.tensor.matmul(ps[c][:], lhsT, rhs).then_inc(s_mm, 1)

    # ---- copies on DVE / ACT (per-engine, in plan order)
    for i in range(len(COPY_PLAN)):
        emit_copy(i)

    nc.finalize()
    _NC_CACHE["nc"] = nc
    return nc


# ------------------------------------------------------------------- wrapper
def _make_in_maps(control_points, knots_x, knots_y):
    cp = np.asarray(control_points, dtype=np.float32)
    A = _dense_basis_matrix(knots_x)                      # (1024, 32) [x, i]
    B = _dense_basis_matrix(knots_y)                      # (1024, 32) [y, j]
    Bt = np.ascontiguousarray(B.T)                        # (32, 1024) [j, y]
    maps = []
    for c in range(NCORES):
        Ac = A[c * ROWS:(c + 1) * ROWS]                   # (128, 32) [x, i]
        # MT[j, d*128+x] = sum_i cp[i,j,d] * Ac[x,i]
        MT = np.einsum("ijd,xi->jdx", cp, Ac).reshape(NCTRL, DIM * ROWS)
        inp = np.concatenate([Bt, MT], axis=1).astype(np.float16)
        maps.append({"inp": inp})
    return maps



def kernel(control_points, knots_x, knots_y):
    from concourse.bass_utils import run_bass_kernel_spmd

    in_maps = _make_in_maps(control_points, knots_x, knots_y)
    nc = _build_nc()
    res = run_bass_kernel_spmd(nc, in_maps, core_ids=list(range(NCORES)))
    # per-core out is [128, 3*1024] fp16, d-major; -> (1024, 1024, 3) f32
    out = np.concatenate([np.asarray(res.results[c]["out"])
                          for c in range(NCORES)], axis=0)
    out = out.reshape(EOUT, DIM, EOUT).transpose(0, 2, 1).astype(np.float32)
    return out.reshape(1, EOUT, EOUT, DIM)
